# revision 64
# baseline (speedup 1.0000x reference)
"""Trainium2 Bass kernel for nn_BiLSTM_CRF_18098992185950 (8 NeuronCores).

Two launches (tolerance is 2e-2; this lands ~2e-5):

  conv(2ch,k3,p1) + Linear(D->1) collapse into fixed 256-d projection
  vectors g_e0/g_e1/g_t0/g_t1 (see _gvec), so all scores are dots of
  embedding rows with 4 fixed vectors. The CRF forward DP in normal space
  is a matrix chain with emit attached to the CURRENT index and factored
  out as a diagonal:
      Z = exp(emit_0)^T (prod_{t=0}^{1022} S_t D_t) 1
      S_t[j,k] = exp(0.5*tanh((u_t[j]+v_{t+1}[k]+ct)/2) + 0.5 + mlogs/2)
      D_t = diag(exp(0.5*tanh((a+b+ce)_{t+1}/2) + 0.5 + mlogs/2))
  (sigmoid written as 0.5*tanh(x/2)+0.5 so tanh and exp share ONE ACT
  table set - no 1.3us table reloads). Emit-on-k keeps every per-(t,k)
  quantity row-major - no layout transposes anywhere on device.

  L1 (memory regime): the host stages the embedding table TRANSPOSED
  (256, V) in fp8-e4m3; each core streams its V/8 shard sequentially
  (3.2 MB) and computes proj = [g_t0|g_t1|g_e1]^T @ tableT with single
  fp8 DoubleRow matmuls (both 128-row k-tiles in one half-rate pass).
  obs is staged transposed too (bf16), giving the a-row the same way.
  (An on-device indirect row gather was tried instead: random 512B DMA
  descriptors run latency-bound at ~1.4 GB/s/engine, 30x slower than
  this sequential stream.)

  Host glue between launches: gathers proj[:, candidate_ids] (~50
  KB/core) and restages it - pure indexing, like the baseline.

  L2 (compute): per core, 128 S-leaves are built two groups at a time
  stacked across all 128 partitions: one K=16 matmul per 8 frames
  against host-staged block-diagonal [u;1]/[diag-ones;v] operands,
  full-width tanh, then exp writes straight into a zeroed block-diagonal
  "leafpair" buffer (pair = two subchains). The chain advances 32
  subchains of 4 leaves as 64 paired 128x128 bf16 matmuls; the PSUM
  evacuation of each round applies D as a free-dim-broadcast multiply.
  The host combines the 8*32 subchain matrices in f64 (as the baseline
  did).
"""

import numpy as np

T = 1024
K = 64
D = 256
V = 100000
NCORES = 8
VSH = 12544          # V-shard columns per core (8*12544 >= V)
NFR = 129            # frames per core (128 + 1 overlap)
NROW = 8320          # staged (t,k) rows: 129*64 = 8256, padded
NG = 16              # leaf groups of 8 frames
NSUB = 32            # subchains per core
LSUB = 4             # leaves per subchain
NBATCH = 4           # chain batches of 8 subchains
LTW = 2048           # L1 stream tile width (columns)

_PROG = {}


def _gvec(w3, l):
    g = np.zeros_like(l)
    g += w3[1] * l
    g[:-1] += w3[0] * l[1:]
    g[1:] += w3[2] * l[:-1]
    return g


def _mods():
    import sys
    if "/opt/trn_rl_repo" not in sys.path:
        sys.path.insert(0, "/opt/trn_rl_repo")
    import concourse.bacc as bacc
    import concourse.mybir as mybir
    from concourse import tile, bass
    return bacc, mybir, tile, bass


def _build_l1():
    if "l1" in _PROG:
        return _PROG["l1"]
    bacc, mybir, tile, bass = _mods()
    f32 = mybir.dt.float32
    bf16 = mybir.dt.bfloat16

    f8 = mybir.dt.float8e4
    nc = bacc.Bacc("TRN2", target_bir_lowering=False, debug=False,
                   enable_asserts=False, num_devices=NCORES)
    ttc = nc.dram_tensor("ttc", (D, VSH), f8, kind="ExternalInput").ap()
    obsT = nc.dram_tensor("obsT", (D, 256), bf16, kind="ExternalInput").ap()
    gmat = nc.dram_tensor("gmat", (D, 16), f8, kind="ExternalInput").ap()
    gobs = nc.dram_tensor("gobs", (D, 1), bf16, kind="ExternalInput").ap()
    projout = nc.dram_tensor("projout", (3, VSH), bf16,
                             kind="ExternalOutput").ap()
    arowout = nc.dram_tensor("arowout", (1, 256), f32,
                             kind="ExternalOutput").ap()

    ttr = ttc.rearrange("(c p) v -> p c v", p=128)
    with tile.TileContext(nc) as tc:
        with (
            tc.tile_pool(name="persist", bufs=1) as pp,
            tc.tile_pool(name="load", bufs=4) as lp,
            tc.tile_pool(name="out", bufs=3) as op,
            tc.tile_pool(name="ps_pj", bufs=4, space="PSUM") as ps_pj,
        ):
            # small inputs go via DVE's DMA queue: SP streams table tiles
            # immediately and ACT's queue is blocked by its table load
            g_sb = pp.tile([128, 2, 16], f8, tag="gmat")
            nc.gpsimd.dma_start(g_sb[:], gmat.rearrange("(c p) g -> p c g", p=128))
            go_sb = pp.tile([128, 2, 1], bf16, tag="gobs")
            nc.gpsimd.dma_start(go_sb[:], gobs.rearrange("(c p) g -> p c g", p=128))

            # a-row from transposed obs: 2 matmuls, no transposes
            obsT_sb = pp.tile([128, 2, 256], bf16, tag="obsT")
            nc.gpsimd.dma_start(obsT_sb[:],
                                obsT.rearrange("(c p) f -> p c f", p=128))
            arow_ps = ps_pj.tile([1, 256], f32, tag="ar")
            for ch in range(2):
                nc.tensor.matmul(
                    out=arow_ps[:], lhsT=go_sb[:, ch, :],
                    rhs=obsT_sb[:, ch, :], start=(ch == 0), stop=(ch == 1),
                )
            arow = pp.tile([1, 256], f32, tag="arow")
            nc.vector.tensor_copy(out=arow[:], in_=arow_ps[:])
            nc.sync.dma_start(out=arowout, in_=arow[:])

            # stream the tableT shard; proj rows [u, v, b].  The first two
            # tiles are small so compute starts as early as possible.
            widths = [1024, 1024] + [2048] * 5 + [256]
            los = [sum(widths[:i]) for i in range(len(widths))]
            for vt, (lo, w) in enumerate(zip(los, widths)):
                tt = lp.tile([128, 2, LTW], f8, tag="tt")
                nc.sync.dma_start(tt[:, :, :w], ttr[:, :, lo:lo + w])
                pr = op.tile([3, LTW], bf16, tag="pr")
                for j in range((w + 511) // 512):
                    w2 = min(512, w - j * 512)
                    pj = ps_pj.tile([16, 512], f32, tag="pj")
                    # fp8 DoubleRow: both 128-row k-tiles contract in one
                    # half-cycle-per-column pass
                    nc.tensor.matmul(
                        out=pj[:, :w2],
                        lhsT=g_sb[:],
                        rhs=tt[:, :, j * 512: j * 512 + w2],
                        start=True, stop=True,
                        perf_mode=mybir.MatmulPerfMode.DoubleRow,
                    )
                    if (vt + j) % 2 == 0:
                        nc.vector.tensor_copy(
                            out=pr[:, j * 512: j * 512 + w2], in_=pj[0:3, :w2])
                    else:
                        nc.scalar.copy(
                            out=pr[:, j * 512: j * 512 + w2], in_=pj[0:3, :w2])
                # issue on ACT's DMA queue: an SP-queued output dma would
                # head-of-line block the next tile's load behind its sem wait
                nc.scalar.dma_start(out=projout[:, lo:lo + w], in_=pr[:, :w])
    nc.compile()
    _PROG["l1"] = nc
    return nc


def _build_l2():
    if "l2" in _PROG:
        return _PROG["l2"]
    bacc, mybir, tile, bass = _mods()
    f32 = mybir.dt.float32
    bf16 = mybir.dt.bfloat16
    AF = mybir.ActivationFunctionType
    OP = mybir.AluOpType

    nc = bacc.Bacc("TRN2", target_bir_lowering=False, debug=False,
                   enable_asserts=False, num_devices=NCORES)
    ul_in = nc.dram_tensor("ulhsT", (2 * 8, NG * K), bf16,
                           kind="ExternalInput").ap()
    vr_in = nc.dram_tensor("vrhs", (2 * 8, NG * 512), bf16,
                           kind="ExternalInput").ap()
    emp_in = nc.dram_tensor("empreT", (128, K), bf16, kind="ExternalInput").ap()
    em0_in = nc.dram_tensor("em0pre", (1, K), f32, kind="ExternalInput").ap()
    cols_in = nc.dram_tensor("cols", (128, 4), f32, kind="ExternalInput").ap()
    addend = nc.dram_tensor("addend", (K, K), bf16, kind="ExternalInput").ap()
    qinit = nc.dram_tensor("qinit", (128, 256), bf16, kind="ExternalInput").ap()
    qout = nc.dram_tensor("qout", (128, NG * K), f32, kind="ExternalOutput").ap()
    em0out = nc.dram_tensor("em0out", (1, K), f32, kind="ExternalOutput").ap()

    with tile.TileContext(nc) as tc:
        with (
            tc.tile_pool(name="persist", bufs=1) as pp,
            tc.tile_pool(name="grp", bufs=3) as gp,
            tc.tile_pool(name="qq", bufs=3) as qp,
            tc.tile_pool(name="ps_pl", bufs=2, space="PSUM") as ps_pl,
            tc.tile_pool(name="ps_pq", bufs=2, space="PSUM") as ps_pq,
        ):
            # the leaf matmul operands gate everything - load them first on SP
            ulhsT0 = pp.tile([16, NG * K], bf16, tag="ulhsT")
            nc.sync.dma_start(ulhsT0[:], ul_in)
            vrhs0 = pp.tile([16, NG * 512], bf16, tag="vrhs")
            nc.sync.dma_start(vrhs0[:], vr_in)

            cols = pp.tile([128, 4], f32, tag="cols")
            nc.scalar.dma_start(cols[:], cols_in)
            ct2_col = cols[:, 0:1]
            cp_col = cols[:, 1:2]       # 0.5 + mlogs/2
            mask_col = cols[:, 2:3]
            maski_col = cols[:, 3:4]    # 1 - mask

            # emit factor as a diagonal: d[k] = exp(0.5*tanh(empre/2) + c'),
            # staged transposed+paired by the host (k on partitions, one
            # column per (pair, round) = leaf)
            empreT = pp.tile([128, K], bf16, tag="empreT")
            nc.scalar.dma_start(empreT[:], emp_in)
            th2d = pp.tile([128, K], bf16, tag="th2d")
            nc.scalar.activation(th2d[:], empreT[:], AF.Tanh, scale=0.5)
            dmat = pp.tile([128, K], bf16, tag="dmat")
            nc.scalar.activation(dmat[:], th2d[:], AF.Exp, bias=cp_col,
                                 scale=0.5)
            # pad-leaf column: d -> 1 on the last core
            nc.vector.scalar_tensor_tensor(
                out=dmat[K:128, 63:64], in0=dmat[K:128, 63:64],
                scalar=mask_col[K:128, :], in1=maski_col[K:128, :],
                op0=OP.mult, op1=OP.add,
            )
            dmv = dmat[:].rearrange("p (m r) -> p m r", r=4)

            ulhsT = ulhsT0
            vrhs = vrhs0
            add_sb2 = pp.tile([128, K], bf16, tag="addend2")
            nc.scalar.dma_start(add_sb2[K:128, :], addend)

            # ---- emit_0 ----
            em0p = pp.tile([1, K], f32, tag="em0p")
            nc.sync.dma_start(em0p[:], em0_in)
            th0 = pp.tile([1, K], f32, tag="th0")
            nc.scalar.activation(th0[:], em0p[:], AF.Tanh, scale=0.5)
            em0sb = pp.tile([1, K], f32, tag="em0")
            nc.vector.tensor_scalar(out=em0sb[:], in0=th0[:], scalar1=0.5,
                                    scalar2=0.5, op0=OP.mult, op1=OP.add)
            nc.sync.dma_start(out=em0out, in_=em0sb[:])

            # zeroed block-diagonal leaf-pair buffer (off-quadrants stay 0).
            # Pair p: top = subchain 4*(p//2)+(p%2) (an even group), bottom =
            # that + 2 (the odd group of the same pair of groups); round r
            # blocks sit at cols (p*4+r)*128, top-left / bottom-right.
            leafpair = pp.tile([128, NG * 512], bf16, tag="leafpair")
            nc.gpsimd.memset(leafpair[:, :NG * 256], 0)
            nc.gpsimd.memset(leafpair[:, NG * 256:], 0)
            lpv = leafpair[:].rearrange("p (m r x) -> p m r x", r=4, x=128)

            # ---- S-leaves: exp(0.5*th1 + c') ----
            # Per pair of groups (2jp -> partitions 0:64, 2jp+1 -> 64:128):
            # two K=16 matmuls against host-staged block-diagonal [u;1] /
            # [diag-ones; v] operands fill one (128,512) psum; tanh runs
            # full-width; exp writes straight into the leafpair quadrants,
            # batched two group-pairs per instruction.
            th1big = pp.tile([128, 8 * 512], bf16, tag="th1big")
            for jp in range(8):
                pl = ps_pl.tile([128, 512], f32, tag="pl")
                nc.tensor.matmul(
                    out=pl[0:K, :],
                    lhsT=ulhsT[:, (2 * jp) * K:(2 * jp + 1) * K],
                    rhs=vrhs[:, (2 * jp) * 512:(2 * jp + 1) * 512],
                    start=True, stop=True,
                )
                nc.tensor.matmul(
                    out=pl[K:128, :],
                    lhsT=ulhsT[:, (2 * jp + 1) * K:(2 * jp + 2) * K],
                    rhs=vrhs[:, (2 * jp + 1) * 512:(2 * jp + 2) * 512],
                    start=True, stop=True,
                )
                nc.scalar.activation(th1big[:, jp * 512:(jp + 1) * 512],
                                     pl[:], AF.Tanh, bias=ct2_col, scale=0.5)
                if jp % 2 == 1:
                    s2v = th1big[:, (jp - 1) * 512:(jp + 1) * 512].rearrange(
                        "p (a r k) -> p a r k", r=4, k=K)
                    p0 = 2 * (jp - 1)
                    if jp < 7:
                        nc.scalar.activation(
                            lpv[0:K, p0:p0 + 4, :, 0:K],
                            s2v[0:K, :, :, :], AF.Exp, bias=cp_col[0:K, :],
                            scale=0.5)
                        nc.scalar.activation(
                            lpv[K:128, p0:p0 + 4, :, K:128],
                            s2v[K:128, :, :, :], AF.Exp, bias=cp_col[K:128, :],
                            scale=0.5)
                    else:
                        # last batch: emit per-round so its chain rounds can
                        # start while later rounds' exps still run
                        for r in range(LSUB):
                            nc.scalar.activation(
                                lpv[0:K, p0:p0 + 4, r, 0:K],
                                s2v[0:K, :, r, :], AF.Exp,
                                bias=cp_col[0:K, :], scale=0.5)
                            nc.scalar.activation(
                                lpv[K:128, p0:p0 + 4, r, K:128],
                                s2v[K:128, :, r, :], AF.Exp,
                                bias=cp_col[K:128, :], scale=0.5)

            # pad leaf 127 (pair 15 bottom, round 3) -> mask*leaf + addend
            last = lpv[K:128, 15, 3, K:128]
            nc.vector.scalar_tensor_tensor(
                out=last, in0=last, scalar=mask_col[K:128, :],
                in1=add_sb2[K:128, :], op0=OP.mult, op1=OP.add,
            )

            # ---- chain: 4 batches of 4 pairs (8 subchains), 4 rounds ----
            # the PSUM->SBUF evacuation applies the emit diagonal: Q <- D S^T Q
            qout_sb = pp.tile([128, NG * K], f32, tag="qout_sb")
            for b in range(NBATCH):
                qcur = qp.tile([128, 256], bf16, tag="q")
                nc.gpsimd.dma_start(qcur[:], qinit)
                for r in range(LSUB):
                    pq = ps_pq.tile([128, 256], f32, tag="pq")
                    for p in range(4):
                        m = 4 * b + p
                        nc.tensor.matmul(
                            out=pq[:, p * K:(p + 1) * K],
                            lhsT=lpv[:, m, r, :],
                            rhs=qcur[:, p * K:(p + 1) * K],
                            start=True, stop=True,
                        )
                    din = dmv[:, 4 * b:4 * b + 4, r].unsqueeze(2).to_broadcast(
                        (128, 4, K))
                    if r < LSUB - 1:
                        qnext = qp.tile([128, 256], bf16, tag="q")
                        dst = qnext[:]
                    else:
                        dst = qout_sb[:, b * 256:(b + 1) * 256]
                    nc.vector.tensor_tensor(
                        out=dst.rearrange("p (q k) -> p q k", k=K),
                        in0=pq[:].rearrange("p (q k) -> p q k", k=K),
                        in1=din, op=OP.mult,
                    )
                    if r < LSUB - 1:
                        qcur = qnext
                # ship each batch's result as soon as it lands
                nc.sync.dma_start(out=qout[:, b * 256:(b + 1) * 256],
                                  in_=qout_sb[:, b * 256:(b + 1) * 256])
    nc.compile()
    _PROG["l2"] = nc
    return nc


def _host_consts(inputs):
    E = np.asarray(inputs["word_embeds"], dtype=np.float32)
    ids = np.asarray(inputs["candidate_ids"]).astype(np.int64)
    obs = np.ascontiguousarray(np.asarray(inputs["observed_feats"], dtype=np.float32))

    lw_e = np.asarray(inputs["emit_lin_w"], dtype=np.float64)[0]
    lw_t = np.asarray(inputs["trans_lin_w"], dtype=np.float64)[0]
    cw_e = np.asarray(inputs["emit_conv_w"], dtype=np.float64)
    cw_t = np.asarray(inputs["trans_conv_w"], dtype=np.float64)
    g_e0 = _gvec(cw_e[0, 0], lw_e)
    g_e1 = _gvec(cw_e[0, 1], lw_e)
    g_t0 = _gvec(cw_t[0, 0], lw_t)
    g_t1 = _gvec(cw_t[0, 1], lw_t)
    ce = float(np.asarray(inputs["emit_conv_b"], np.float64)[0] * lw_e.sum()
               + np.asarray(inputs["emit_lin_b"], np.float64)[0])
    ct = float(np.asarray(inputs["trans_conv_b"], np.float64)[0] * lw_t.sum()
               + np.asarray(inputs["trans_lin_b"], np.float64)[0])

    samp = E[ids[:8].ravel()].astype(np.float64)
    sig = 1.0 / (1.0 + np.exp(-((samp @ g_t0).mean() + (samp @ g_t1).mean() + ct)))
    a8 = obs[:8].astype(np.float64) @ g_e0
    em = 1.0 / (1.0 + np.exp(-(a8.mean() + (samp @ g_e1).mean() + ce)))
    s = float(64.0 * np.exp(sig + em))
    gmat = np.stack([g_t0, g_t1, g_e1, g_e0, np.zeros(D)], axis=1)
    return E, ids, obs, gmat, ce, ct, s


def _run_launches(inputs, run_kw1=None, run_kw2=None):
    import sys
    if "/opt/trn_rl_repo" not in sys.path:
        sys.path.insert(0, "/opt/trn_rl_repo")
    from concourse.bass_utils import run_bass_kernel_spmd
    import ml_dtypes

    bf16 = ml_dtypes.bfloat16
    run_kw1 = run_kw1 or {}
    run_kw2 = run_kw2 or {}
    E, ids, obs, gmat, ce, ct, s = _host_consts(inputs)
    mlogs = -np.log(s)

    # ---- L1: stream transposed table shards (fp8) ----
    f8 = ml_dtypes.float8_e4m3
    tT = np.zeros((D, NCORES * VSH), dtype=f8)
    tT[:, :V] = np.ascontiguousarray(E.T.astype(f8))
    obsTf = np.zeros((D, T + 128), dtype=bf16)
    obsTf[:, :T] = np.ascontiguousarray(obs.T.astype(bf16))
    gmb = np.ascontiguousarray(
        np.concatenate([gmat[:, 0:3], np.zeros((D, 13))], axis=1)
        .astype(np.float32).astype(f8))
    gob = np.ascontiguousarray(gmat[:, 3:4].astype(np.float32).astype(bf16))

    l1 = _build_l1()
    in1 = [{
        "ttc": np.ascontiguousarray(tT[:, c * VSH:(c + 1) * VSH]),
        "obsT": np.ascontiguousarray(obsTf[:, 128 * c: 128 * c + 256]),
        "gmat": gmb,
        "gobs": gob,
    } for c in range(NCORES)]
    res1 = run_bass_kernel_spmd(l1, in1, core_ids=list(range(NCORES)),
                                **run_kw1)
    proj = np.concatenate(
        [res1.results[c]["projout"] for c in range(NCORES)], axis=1)[:, :V]

    # ---- host gather + staging (indexing glue) ----
    ids_pad = np.zeros((T + 1, K), dtype=np.int64)
    ids_pad[:T] = ids
    l2 = _build_l2()
    eye_s = (np.eye(K, dtype=np.float64) / s).astype(np.float32).astype(bf16)
    zer = np.zeros((K, K), dtype=bf16)
    eye2 = np.vstack([np.eye(K, dtype=np.float32)] * 2)
    qinitb = np.ascontiguousarray(np.tile(eye2, (1, 4)).astype(bf16))
    in2 = []
    for c in range(NCORES):
        fr0 = 128 * c
        rid = ids_pad[fr0:fr0 + NFR].ravel()          # 8256
        pg = proj[:, rid]                              # (3, 8256) bf16
        u = pg[0].astype(np.float32)
        v = pg[1].astype(np.float32)
        # block-diagonal outer-sum operands, one K=16 matmul per 8 frames
        ulhsT = np.zeros((16, NG * K), dtype=np.float32)
        vrhs = np.zeros((16, NG * 512), dtype=np.float32)
        ur = u[:8192].reshape(NG, 8, K)
        vr = v[K:8256].reshape(NG, 8, K)               # frames 1..128
        vrh = vrhs.reshape(16, NG, 8, K)
        ulh = ulhsT.reshape(16, NG, K)
        for q in range(8):
            ulh[2 * q] = ur[:, q, :]
            ulh[2 * q + 1] = 1.0
            vrh[2 * q, :, q, :] = 1.0
            vrh[2 * q + 1, :, q, :] = vr[:, q, :]
        # emit pre-activations: restage L1's b/a outputs with the +ce fold.
        # empreT[(h*64+k), p*4+r] = empre of the leaf at (pair p, round r,
        # half h), where pair p tops subchain 4*(p//2)+(p%2), bottom +2.
        arow_c = res1.results[c]["arowout"][0].astype(np.float32)
        bfl = pg[2].astype(np.float32)                 # (8256,) b dots
        empre = (bfl[K:8256].reshape(128, K)
                 + arow_c[1:129, None] + np.float32(ce))   # (128 leaves, K)
        empreT = np.zeros((128, K), dtype=np.float32)
        for p in range(16):
            sc_t = 4 * (p // 2) + (p % 2)
            for r in range(4):
                empreT[0:K, p * 4 + r] = empre[4 * sc_t + r]
                empreT[K:128, p * 4 + r] = empre[4 * (sc_t + 2) + r]
        em0pre = (bfl[:K] + arow_c[0] + np.float32(ce)).reshape(1, K)
        colsv = np.zeros((128, 4), dtype=np.float32)
        colsv[:, 0] = ct / 2.0
        colsv[:, 1] = 0.5 + mlogs / 2.0
        colsv[:, 2] = 0.0 if c == NCORES - 1 else 1.0
        colsv[:, 3] = 1.0 - colsv[:, 2]
        in2.append({
            "ulhsT": np.ascontiguousarray(ulhsT.astype(bf16)),
            "vrhs": np.ascontiguousarray(vrhs.astype(bf16)),
            "empreT": np.ascontiguousarray(empreT.astype(bf16)),
            "em0pre": np.ascontiguousarray(em0pre),
            "cols": colsv,
            "addend": eye_s if c == NCORES - 1 else zer,
            "qinit": qinitb,
        })
    res2 = run_bass_kernel_spmd(l2, in2, core_ids=list(range(NCORES)),
                                **run_kw2)

    # ---- host combine in f64 ----
    # qout is (128, 16*64): pair p (cols p*64) holds subchain 4*(p//2)+(p%2)
    # in rows 0:64 and that + 2 in rows 64:128
    P = np.eye(K, dtype=np.float64)
    acc = 0.0
    for c in range(NCORES):
        qo = res2.results[c]["qout"].astype(np.float64)
        for sc in range(NSUB):
            blk, rem = divmod(sc, 4)
            h, p = (0, 2 * blk + rem) if rem < 2 else (1, 2 * blk + rem - 2)
            Q = qo[h * K:(h + 1) * K, p * K:(p + 1) * K]
            P = P @ Q.T
            mx = np.abs(P).max()
            P /= mx
            acc += np.log(mx)
    em0 = res2.results[0]["em0out"][0].astype(np.float64)
    z = np.exp(em0) @ P @ np.ones(K)
    ans = np.log(z) + acc + NSUB * LSUB * NCORES * np.log(np.float64(s))
    return np.array([ans], dtype=np.float32), res1, res2


def kernel(**inputs):
    ans, _, _ = _run_launches(inputs)
    return ans


def profiled_run(inputs):
    """Run both launches with NTFF tracing; return summed exec ns (or None)."""
    import sys as _sys
    import types as _types
    try:
        if "antenv.axon_hooks" not in _sys.modules:
            from trn_agent_boot.trn_boot import _ntff_profile_via_ctypes
            hook = _ntff_profile_via_ctypes("/opt/axon/libaxon_pjrt.so")
            mod = _types.ModuleType("antenv.axon_hooks")
            mod.get_axon_ntff_profile_hook = lambda: hook
            mod.set_axon_ntff_profile_hook = lambda h: None
            _sys.modules["antenv.axon_hooks"] = mod
            import antenv
            antenv.axon_hooks = mod
    except Exception as e:
        print(f"profile shim unavailable: {e}")
        return None
    kw = {"trace": True, "trace_cores": [0]}
    ans, res1, res2 = _run_launches(inputs, run_kw1=dict(kw), run_kw2=dict(kw))
    print("profiled answer:", ans)
    for name, r in (("L1", res1), ("L2", res2)):
        tr = r.instructions_and_trace
        print(f"{name}: exec_time_ns={r.exec_time_ns}"
              + (f" trace={tr[1]}" if tr else ""))
    if res1.exec_time_ns is None or res2.exec_time_ns is None:
        return None
    return res1.exec_time_ns + res2.exec_time_ns


# revision 66
# speedup vs baseline: 1.1089x; 1.1089x over previous
"""Trainium2 Bass kernel for nn_BiLSTM_CRF_18098992185950 (8 NeuronCores).

Two launches (tolerance is 2e-2; this lands ~2e-5):

  conv(2ch,k3,p1) + Linear(D->1) collapse into fixed 256-d projection
  vectors g_e0/g_e1/g_t0/g_t1 (see _gvec), so all scores are dots of
  embedding rows with 4 fixed vectors. The CRF forward DP in normal space
  is a matrix chain with emit attached to the CURRENT index and factored
  out as a diagonal:
      Z = exp(emit_0)^T (prod_{t=0}^{1022} S_t D_t) 1
      S_t[j,k] = exp(0.5*tanh((u_t[j]+v_{t+1}[k]+ct)/2) + 0.5 + mlogs/2)
      D_t = diag(exp(0.5*tanh((a+b+ce)_{t+1}/2) + 0.5 + mlogs/2))
  (sigmoid written as 0.5*tanh(x/2)+0.5 so tanh and exp share ONE ACT
  table set - no 1.3us table reloads). Emit-on-k keeps every per-(t,k)
  quantity row-major - no layout transposes anywhere on device.

  L1 (memory regime): the host stages the embedding table TRANSPOSED
  (256, V) in fp8-e4m3; each core streams its V/8 shard sequentially
  (3.2 MB) and computes proj = [g_t0|g_t1|g_e1]^T @ tableT with single
  fp8 DoubleRow matmuls (both 128-row k-tiles in one half-rate pass).
  obs is staged transposed too (bf16), giving the a-row the same way.
  (An on-device indirect row gather was tried instead: random 512B DMA
  descriptors run latency-bound at ~1.4 GB/s/engine, 30x slower than
  this sequential stream.)

  Host glue between launches: gathers proj[:, candidate_ids] (~50
  KB/core) and restages it - pure indexing, like the baseline.

  L2 (compute): per core, 128 S-leaves are built two groups at a time
  stacked across all 128 partitions: one K=16 matmul per 8 frames
  against host-staged block-diagonal [u;1]/[diag-ones;v] operands,
  full-width tanh, then exp writes straight into a zeroed block-diagonal
  "leafpair" buffer (pair = two subchains). The chain advances 32
  subchains of 4 leaves as 64 paired 128x128 bf16 matmuls; the PSUM
  evacuation of each round applies D as a free-dim-broadcast multiply.
  The host combines the 8*32 subchain matrices in f64 (as the baseline
  did).
"""

import numpy as np

T = 1024
K = 64
D = 256
V = 100000
NCORES = 8
VSH = 12544          # V-shard columns per core (8*12544 >= V)
NFR = 129            # frames per core (128 + 1 overlap)
NROW = 8320          # staged (t,k) rows: 129*64 = 8256, padded
NG = 16              # leaf groups of 8 frames
NSUB = 32            # subchains per core
LSUB = 4             # leaves per subchain
NBATCH = 4           # chain batches of 8 subchains
LTW = 2048           # L1 stream tile width (columns)

_PROG = {}


def _gvec(w3, l):
    g = np.zeros_like(l)
    g += w3[1] * l
    g[:-1] += w3[0] * l[1:]
    g[1:] += w3[2] * l[:-1]
    return g


def _mods():
    import sys
    if "/opt/trn_rl_repo" not in sys.path:
        sys.path.insert(0, "/opt/trn_rl_repo")
    import concourse.bacc as bacc
    import concourse.mybir as mybir
    from concourse import tile, bass
    return bacc, mybir, tile, bass


def _build_l1():
    if "l1" in _PROG:
        return _PROG["l1"]
    bacc, mybir, tile, bass = _mods()
    f32 = mybir.dt.float32
    bf16 = mybir.dt.bfloat16

    f8 = mybir.dt.float8e4
    nc = bacc.Bacc("TRN2", target_bir_lowering=False, debug=False,
                   enable_asserts=False, num_devices=NCORES)
    ttc = nc.dram_tensor("ttc", (D, VSH), f8, kind="ExternalInput").ap()
    obsT = nc.dram_tensor("obsT", (D, 256), bf16, kind="ExternalInput").ap()
    gmat = nc.dram_tensor("gmat", (D, 16), f8, kind="ExternalInput").ap()
    gobs = nc.dram_tensor("gobs", (D, 1), bf16, kind="ExternalInput").ap()
    projout = nc.dram_tensor("projout", (3, VSH), bf16,
                             kind="ExternalOutput").ap()
    arowout = nc.dram_tensor("arowout", (1, 256), f32,
                             kind="ExternalOutput").ap()

    ttr = ttc.rearrange("(c p) v -> p c v", p=128)
    with tile.TileContext(nc) as tc:
        with (
            tc.tile_pool(name="persist", bufs=1) as pp,
            tc.tile_pool(name="load", bufs=8) as lp,
            tc.tile_pool(name="out", bufs=3) as op,
            tc.tile_pool(name="ps_pj", bufs=4, space="PSUM") as ps_pj,
        ):
            # small inputs go via DVE's DMA queue: SP streams table tiles
            # immediately and ACT's queue is blocked by its table load
            g_sb = pp.tile([128, 2, 16], f8, tag="gmat")
            nc.gpsimd.dma_start(g_sb[:], gmat.rearrange("(c p) g -> p c g", p=128))
            go_sb = pp.tile([128, 2, 1], bf16, tag="gobs")
            nc.gpsimd.dma_start(go_sb[:], gobs.rearrange("(c p) g -> p c g", p=128))

            # a-row from transposed obs: 2 matmuls, no transposes
            obsT_sb = pp.tile([128, 2, 256], bf16, tag="obsT")
            nc.gpsimd.dma_start(obsT_sb[:],
                                obsT.rearrange("(c p) f -> p c f", p=128))
            arow_ps = ps_pj.tile([1, 256], f32, tag="ar")
            for ch in range(2):
                nc.tensor.matmul(
                    out=arow_ps[:], lhsT=go_sb[:, ch, :],
                    rhs=obsT_sb[:, ch, :], start=(ch == 0), stop=(ch == 1),
                )
            arow = pp.tile([1, 256], f32, tag="arow")
            nc.vector.tensor_copy(out=arow[:], in_=arow_ps[:])
            nc.sync.dma_start(out=arowout, in_=arow[:])

            # stream the tableT shard; proj rows [u, v, b].  The first two
            # tiles are small so compute starts as early as possible.
            widths = [1024, 1024] + [2048] * 5 + [256]
            los = [sum(widths[:i]) for i in range(len(widths))]
            for vt, (lo, w) in enumerate(zip(los, widths)):
                tt = lp.tile([128, 2, LTW], f8, tag="tt")
                nc.sync.dma_start(tt[:, :, :w], ttr[:, :, lo:lo + w])
                pr = op.tile([3, LTW], bf16, tag="pr")
                for j in range((w + 511) // 512):
                    w2 = min(512, w - j * 512)
                    pj = ps_pj.tile([16, 512], f32, tag="pj")
                    # fp8 DoubleRow: both 128-row k-tiles contract in one
                    # half-cycle-per-column pass
                    nc.tensor.matmul(
                        out=pj[:, :w2],
                        lhsT=g_sb[:],
                        rhs=tt[:, :, j * 512: j * 512 + w2],
                        start=True, stop=True,
                        perf_mode=mybir.MatmulPerfMode.DoubleRow,
                    )
                    if (vt + j) % 2 == 0:
                        nc.vector.tensor_copy(
                            out=pr[:, j * 512: j * 512 + w2], in_=pj[0:3, :w2])
                    else:
                        nc.scalar.copy(
                            out=pr[:, j * 512: j * 512 + w2], in_=pj[0:3, :w2])
                # issue on the idle Pool queue: SP would head-of-line block
                # the next tile load, ACT would stall the evacuations
                nc.gpsimd.dma_start(out=projout[:, lo:lo + w], in_=pr[:, :w])
    nc.compile()
    _PROG["l1"] = nc
    return nc


def _build_l2():
    if "l2" in _PROG:
        return _PROG["l2"]
    bacc, mybir, tile, bass = _mods()
    f32 = mybir.dt.float32
    bf16 = mybir.dt.bfloat16
    AF = mybir.ActivationFunctionType
    OP = mybir.AluOpType

    nc = bacc.Bacc("TRN2", target_bir_lowering=False, debug=False,
                   enable_asserts=False, num_devices=NCORES)
    ul_in = nc.dram_tensor("ulhsT", (2 * 8, NG * K), bf16,
                           kind="ExternalInput").ap()
    vr_in = nc.dram_tensor("vrhs", (2 * 8, NG * 512), bf16,
                           kind="ExternalInput").ap()
    emp_in = nc.dram_tensor("empreT", (128, K), bf16, kind="ExternalInput").ap()
    em0_in = nc.dram_tensor("em0pre", (1, K), f32, kind="ExternalInput").ap()
    cols_in = nc.dram_tensor("cols", (128, 4), f32, kind="ExternalInput").ap()
    addend = nc.dram_tensor("addend", (K, K), bf16, kind="ExternalInput").ap()
    qinit = nc.dram_tensor("qinit", (128, 256), bf16, kind="ExternalInput").ap()
    qout = nc.dram_tensor("qout", (128, NG * K), f32, kind="ExternalOutput").ap()
    em0out = nc.dram_tensor("em0out", (1, K), f32, kind="ExternalOutput").ap()

    with tile.TileContext(nc) as tc:
        with (
            tc.tile_pool(name="persist", bufs=1) as pp,
            tc.tile_pool(name="grp", bufs=3) as gp,
            tc.tile_pool(name="qq", bufs=3) as qp,
            tc.tile_pool(name="ps_pl", bufs=2, space="PSUM") as ps_pl,
            tc.tile_pool(name="ps_pq", bufs=2, space="PSUM") as ps_pq,
        ):
            # the leaf matmul operands gate everything - load them first on SP
            ulhsT0 = pp.tile([16, NG * K], bf16, tag="ulhsT")
            nc.sync.dma_start(ulhsT0[:], ul_in)
            vrhs0 = pp.tile([16, NG * 512], bf16, tag="vrhs")
            for c4 in range(4):
                nc.sync.dma_start(vrhs0[:, c4 * 2048:(c4 + 1) * 2048],
                                  vr_in[:, c4 * 2048:(c4 + 1) * 2048])

            cols = pp.tile([128, 4], f32, tag="cols")
            nc.scalar.dma_start(cols[:], cols_in)
            ct2_col = cols[:, 0:1]
            cp_col = cols[:, 1:2]       # 0.5 + mlogs/2
            mask_col = cols[:, 2:3]
            maski_col = cols[:, 3:4]    # 1 - mask

            # emit factor as a diagonal: d[k] = exp(0.5*tanh(empre/2) + c'),
            # staged transposed+paired by the host (k on partitions, one
            # column per (pair, round) = leaf)
            empreT = pp.tile([128, K], bf16, tag="empreT")
            nc.scalar.dma_start(empreT[:], emp_in)
            th2d = pp.tile([128, K], bf16, tag="th2d")
            nc.scalar.activation(th2d[:], empreT[:], AF.Tanh, scale=0.5)
            dmat = pp.tile([128, K], bf16, tag="dmat")
            nc.scalar.activation(dmat[:], th2d[:], AF.Exp, bias=cp_col,
                                 scale=0.5)
            # pad-leaf column: d -> 1 on the last core
            nc.vector.scalar_tensor_tensor(
                out=dmat[K:128, 63:64], in0=dmat[K:128, 63:64],
                scalar=mask_col[K:128, :], in1=maski_col[K:128, :],
                op0=OP.mult, op1=OP.add,
            )
            dmv = dmat[:].rearrange("p (m r) -> p m r", r=4)

            ulhsT = ulhsT0
            vrhs = vrhs0
            add_sb2 = pp.tile([128, K], bf16, tag="addend2")
            nc.scalar.dma_start(add_sb2[K:128, :], addend)

            # ---- emit_0 ----
            em0p = pp.tile([1, K], f32, tag="em0p")
            nc.sync.dma_start(em0p[:], em0_in)
            th0 = pp.tile([1, K], f32, tag="th0")
            nc.scalar.activation(th0[:], em0p[:], AF.Tanh, scale=0.5)
            em0sb = pp.tile([1, K], f32, tag="em0")
            nc.vector.tensor_scalar(out=em0sb[:], in0=th0[:], scalar1=0.5,
                                    scalar2=0.5, op0=OP.mult, op1=OP.add)
            nc.sync.dma_start(out=em0out, in_=em0sb[:])

            # zeroed block-diagonal leaf-pair buffer (off-quadrants stay 0).
            # Pair p: top = subchain 4*(p//2)+(p%2) (an even group), bottom =
            # that + 2 (the odd group of the same pair of groups); round r
            # blocks sit at cols (p*4+r)*128, top-left / bottom-right.
            leafpair = pp.tile([128, NG * 512], bf16, tag="leafpair")
            nc.gpsimd.memset(leafpair[:, :NG * 256], 0)
            nc.gpsimd.memset(leafpair[:, NG * 256:], 0)
            lpv = leafpair[:].rearrange("p (m r x) -> p m r x", r=4, x=128)

            # ---- S-leaves: exp(0.5*th1 + c') ----
            # Per pair of groups (2jp -> partitions 0:64, 2jp+1 -> 64:128):
            # two K=16 matmuls against host-staged block-diagonal [u;1] /
            # [diag-ones; v] operands fill one (128,512) psum; tanh runs
            # full-width; exp writes straight into the leafpair quadrants,
            # batched two group-pairs per instruction.
            th1big = pp.tile([128, 8 * 512], bf16, tag="th1big")
            for jp in range(8):
                pl = ps_pl.tile([128, 512], f32, tag="pl")
                nc.tensor.matmul(
                    out=pl[0:K, :],
                    lhsT=ulhsT[:, (2 * jp) * K:(2 * jp + 1) * K],
                    rhs=vrhs[:, (2 * jp) * 512:(2 * jp + 1) * 512],
                    start=True, stop=True,
                )
                nc.tensor.matmul(
                    out=pl[K:128, :],
                    lhsT=ulhsT[:, (2 * jp + 1) * K:(2 * jp + 2) * K],
                    rhs=vrhs[:, (2 * jp + 1) * 512:(2 * jp + 2) * 512],
                    start=True, stop=True,
                )
                nc.scalar.activation(th1big[:, jp * 512:(jp + 1) * 512],
                                     pl[:], AF.Tanh, bias=ct2_col, scale=0.5)
                if jp % 2 == 1:
                    s2v = th1big[:, (jp - 1) * 512:(jp + 1) * 512].rearrange(
                        "p (a r k) -> p a r k", r=4, k=K)
                    p0 = 2 * (jp - 1)
                    if jp < 7:
                        nc.scalar.activation(
                            lpv[0:K, p0:p0 + 4, :, 0:K],
                            s2v[0:K, :, :, :], AF.Exp, bias=cp_col[0:K, :],
                            scale=0.5)
                        nc.scalar.activation(
                            lpv[K:128, p0:p0 + 4, :, K:128],
                            s2v[K:128, :, :, :], AF.Exp, bias=cp_col[K:128, :],
                            scale=0.5)
                    else:
                        # last batch: emit per-round so its chain rounds can
                        # start while later rounds' exps still run
                        for r in range(LSUB):
                            nc.scalar.activation(
                                lpv[0:K, p0:p0 + 4, r, 0:K],
                                s2v[0:K, :, r, :], AF.Exp,
                                bias=cp_col[0:K, :], scale=0.5)
                            nc.scalar.activation(
                                lpv[K:128, p0:p0 + 4, r, K:128],
                                s2v[K:128, :, r, :], AF.Exp,
                                bias=cp_col[K:128, :], scale=0.5)

            # pad leaf 127 (pair 15 bottom, round 3) -> mask*leaf + addend
            last = lpv[K:128, 15, 3, K:128]
            nc.vector.scalar_tensor_tensor(
                out=last, in0=last, scalar=mask_col[K:128, :],
                in1=add_sb2[K:128, :], op0=OP.mult, op1=OP.add,
            )

            # ---- chain: 4 batches of 4 pairs (8 subchains), 4 rounds ----
            # the PSUM->SBUF evacuation applies the emit diagonal: Q <- D S^T Q
            qout_sb = pp.tile([128, NG * K], f32, tag="qout_sb")
            for b in range(NBATCH):
                qcur = qp.tile([128, 256], bf16, tag="q")
                nc.gpsimd.dma_start(qcur[:], qinit)
                for r in range(LSUB):
                    pq = ps_pq.tile([128, 256], f32, tag="pq")
                    for p in range(4):
                        m = 4 * b + p
                        nc.tensor.matmul(
                            out=pq[:, p * K:(p + 1) * K],
                            lhsT=lpv[:, m, r, :],
                            rhs=qcur[:, p * K:(p + 1) * K],
                            start=True, stop=True,
                        )
                    din = dmv[:, 4 * b:4 * b + 4, r].unsqueeze(2).to_broadcast(
                        (128, 4, K))
                    if r < LSUB - 1:
                        qnext = qp.tile([128, 256], bf16, tag="q")
                        dst = qnext[:]
                    else:
                        dst = qout_sb[:, b * 256:(b + 1) * 256]
                    nc.vector.tensor_tensor(
                        out=dst.rearrange("p (q k) -> p q k", k=K),
                        in0=pq[:].rearrange("p (q k) -> p q k", k=K),
                        in1=din, op=OP.mult,
                    )
                    if r < LSUB - 1:
                        qcur = qnext
            nc.sync.dma_start(out=qout, in_=qout_sb[:])
    nc.compile()
    _PROG["l2"] = nc
    return nc


def _host_consts(inputs):
    E = np.asarray(inputs["word_embeds"], dtype=np.float32)
    ids = np.asarray(inputs["candidate_ids"]).astype(np.int64)
    obs = np.ascontiguousarray(np.asarray(inputs["observed_feats"], dtype=np.float32))

    lw_e = np.asarray(inputs["emit_lin_w"], dtype=np.float64)[0]
    lw_t = np.asarray(inputs["trans_lin_w"], dtype=np.float64)[0]
    cw_e = np.asarray(inputs["emit_conv_w"], dtype=np.float64)
    cw_t = np.asarray(inputs["trans_conv_w"], dtype=np.float64)
    g_e0 = _gvec(cw_e[0, 0], lw_e)
    g_e1 = _gvec(cw_e[0, 1], lw_e)
    g_t0 = _gvec(cw_t[0, 0], lw_t)
    g_t1 = _gvec(cw_t[0, 1], lw_t)
    ce = float(np.asarray(inputs["emit_conv_b"], np.float64)[0] * lw_e.sum()
               + np.asarray(inputs["emit_lin_b"], np.float64)[0])
    ct = float(np.asarray(inputs["trans_conv_b"], np.float64)[0] * lw_t.sum()
               + np.asarray(inputs["trans_lin_b"], np.float64)[0])

    samp = E[ids[:8].ravel()].astype(np.float64)
    sig = 1.0 / (1.0 + np.exp(-((samp @ g_t0).mean() + (samp @ g_t1).mean() + ct)))
    a8 = obs[:8].astype(np.float64) @ g_e0
    em = 1.0 / (1.0 + np.exp(-(a8.mean() + (samp @ g_e1).mean() + ce)))
    s = float(64.0 * np.exp(sig + em))
    gmat = np.stack([g_t0, g_t1, g_e1, g_e0, np.zeros(D)], axis=1)
    return E, ids, obs, gmat, ce, ct, s


def _run_launches(inputs, run_kw1=None, run_kw2=None):
    import sys
    if "/opt/trn_rl_repo" not in sys.path:
        sys.path.insert(0, "/opt/trn_rl_repo")
    from concourse.bass_utils import run_bass_kernel_spmd
    import ml_dtypes

    bf16 = ml_dtypes.bfloat16
    run_kw1 = run_kw1 or {}
    run_kw2 = run_kw2 or {}
    E, ids, obs, gmat, ce, ct, s = _host_consts(inputs)
    mlogs = -np.log(s)

    # ---- L1: stream transposed table shards (fp8) ----
    f8 = ml_dtypes.float8_e4m3
    tT = np.zeros((D, NCORES * VSH), dtype=f8)
    tT[:, :V] = np.ascontiguousarray(E.T.astype(f8))
    obsTf = np.zeros((D, T + 128), dtype=bf16)
    obsTf[:, :T] = np.ascontiguousarray(obs.T.astype(bf16))
    gmb = np.ascontiguousarray(
        np.concatenate([gmat[:, 0:3], np.zeros((D, 13))], axis=1)
        .astype(np.float32).astype(f8))
    gob = np.ascontiguousarray(gmat[:, 3:4].astype(np.float32).astype(bf16))

    l1 = _build_l1()
    in1 = [{
        "ttc": np.ascontiguousarray(tT[:, c * VSH:(c + 1) * VSH]),
        "obsT": np.ascontiguousarray(obsTf[:, 128 * c: 128 * c + 256]),
        "gmat": gmb,
        "gobs": gob,
    } for c in range(NCORES)]
    res1 = run_bass_kernel_spmd(l1, in1, core_ids=list(range(NCORES)),
                                **run_kw1)
    proj = np.concatenate(
        [res1.results[c]["projout"] for c in range(NCORES)], axis=1)[:, :V]

    # ---- host gather + staging (indexing glue) ----
    ids_pad = np.zeros((T + 1, K), dtype=np.int64)
    ids_pad[:T] = ids
    l2 = _build_l2()
    eye_s = (np.eye(K, dtype=np.float64) / s).astype(np.float32).astype(bf16)
    zer = np.zeros((K, K), dtype=bf16)
    eye2 = np.vstack([np.eye(K, dtype=np.float32)] * 2)
    qinitb = np.ascontiguousarray(np.tile(eye2, (1, 4)).astype(bf16))
    in2 = []
    for c in range(NCORES):
        fr0 = 128 * c
        rid = ids_pad[fr0:fr0 + NFR].ravel()          # 8256
        pg = proj[:, rid]                              # (3, 8256) bf16
        u = pg[0].astype(np.float32)
        v = pg[1].astype(np.float32)
        # block-diagonal outer-sum operands, one K=16 matmul per 8 frames
        ulhsT = np.zeros((16, NG * K), dtype=np.float32)
        vrhs = np.zeros((16, NG * 512), dtype=np.float32)
        ur = u[:8192].reshape(NG, 8, K)
        vr = v[K:8256].reshape(NG, 8, K)               # frames 1..128
        vrh = vrhs.reshape(16, NG, 8, K)
        ulh = ulhsT.reshape(16, NG, K)
        for q in range(8):
            ulh[2 * q] = ur[:, q, :]
            ulh[2 * q + 1] = 1.0
            vrh[2 * q, :, q, :] = 1.0
            vrh[2 * q + 1, :, q, :] = vr[:, q, :]
        # emit pre-activations: restage L1's b/a outputs with the +ce fold.
        # empreT[(h*64+k), p*4+r] = empre of the leaf at (pair p, round r,
        # half h), where pair p tops subchain 4*(p//2)+(p%2), bottom +2.
        arow_c = res1.results[c]["arowout"][0].astype(np.float32)
        bfl = pg[2].astype(np.float32)                 # (8256,) b dots
        empre = (bfl[K:8256].reshape(128, K)
                 + arow_c[1:129, None] + np.float32(ce))   # (128 leaves, K)
        empreT = np.zeros((128, K), dtype=np.float32)
        for p in range(16):
            sc_t = 4 * (p // 2) + (p % 2)
            for r in range(4):
                empreT[0:K, p * 4 + r] = empre[4 * sc_t + r]
                empreT[K:128, p * 4 + r] = empre[4 * (sc_t + 2) + r]
        em0pre = (bfl[:K] + arow_c[0] + np.float32(ce)).reshape(1, K)
        colsv = np.zeros((128, 4), dtype=np.float32)
        colsv[:, 0] = ct / 2.0
        colsv[:, 1] = 0.5 + mlogs / 2.0
        colsv[:, 2] = 0.0 if c == NCORES - 1 else 1.0
        colsv[:, 3] = 1.0 - colsv[:, 2]
        in2.append({
            "ulhsT": np.ascontiguousarray(ulhsT.astype(bf16)),
            "vrhs": np.ascontiguousarray(vrhs.astype(bf16)),
            "empreT": np.ascontiguousarray(empreT.astype(bf16)),
            "em0pre": np.ascontiguousarray(em0pre),
            "cols": colsv,
            "addend": eye_s if c == NCORES - 1 else zer,
            "qinit": qinitb,
        })
    res2 = run_bass_kernel_spmd(l2, in2, core_ids=list(range(NCORES)),
                                **run_kw2)

    # ---- host combine in f64 ----
    # qout is (128, 16*64): pair p (cols p*64) holds subchain 4*(p//2)+(p%2)
    # in rows 0:64 and that + 2 in rows 64:128
    P = np.eye(K, dtype=np.float64)
    acc = 0.0
    for c in range(NCORES):
        qo = res2.results[c]["qout"].astype(np.float64)
        for sc in range(NSUB):
            blk, rem = divmod(sc, 4)
            h, p = (0, 2 * blk + rem) if rem < 2 else (1, 2 * blk + rem - 2)
            Q = qo[h * K:(h + 1) * K, p * K:(p + 1) * K]
            P = P @ Q.T
            mx = np.abs(P).max()
            P /= mx
            acc += np.log(mx)
    em0 = res2.results[0]["em0out"][0].astype(np.float64)
    z = np.exp(em0) @ P @ np.ones(K)
    ans = np.log(z) + acc + NSUB * LSUB * NCORES * np.log(np.float64(s))
    return np.array([ans], dtype=np.float32), res1, res2


def kernel(**inputs):
    ans, _, _ = _run_launches(inputs)
    return ans


def profiled_run(inputs):
    """Run both launches with NTFF tracing; return summed exec ns (or None)."""
    import sys as _sys
    import types as _types
    try:
        if "antenv.axon_hooks" not in _sys.modules:
            from trn_agent_boot.trn_boot import _ntff_profile_via_ctypes
            hook = _ntff_profile_via_ctypes("/opt/axon/libaxon_pjrt.so")
            mod = _types.ModuleType("antenv.axon_hooks")
            mod.get_axon_ntff_profile_hook = lambda: hook
            mod.set_axon_ntff_profile_hook = lambda h: None
            _sys.modules["antenv.axon_hooks"] = mod
            import antenv
            antenv.axon_hooks = mod
    except Exception as e:
        print(f"profile shim unavailable: {e}")
        return None
    kw = {"trace": True, "trace_cores": [0]}
    ans, res1, res2 = _run_launches(inputs, run_kw1=dict(kw), run_kw2=dict(kw))
    print("profiled answer:", ans)
    for name, r in (("L1", res1), ("L2", res2)):
        tr = r.instructions_and_trace
        print(f"{name}: exec_time_ns={r.exec_time_ns}"
              + (f" trace={tr[1]}" if tr else ""))
    if res1.exec_time_ns is None or res2.exec_time_ns is None:
        return None
    return res1.exec_time_ns + res2.exec_time_ns


# revision 68
# speedup vs baseline: 1.1624x; 1.0483x over previous
"""Trainium2 Bass kernel for nn_BiLSTM_CRF_18098992185950 (8 NeuronCores).

Two launches (tolerance is 2e-2; this lands ~2e-5):

  conv(2ch,k3,p1) + Linear(D->1) collapse into fixed 256-d projection
  vectors g_e0/g_e1/g_t0/g_t1 (see _gvec), so all scores are dots of
  embedding rows with 4 fixed vectors. The CRF forward DP in normal space
  is a matrix chain with emit attached to the CURRENT index and factored
  out as a diagonal:
      Z = exp(emit_0)^T (prod_{t=0}^{1022} S_t D_t) 1
      S_t[j,k] = exp(0.5*tanh((u_t[j]+v_{t+1}[k]+ct)/2) + 0.5 + mlogs/2)
      D_t = diag(exp(0.5*tanh((a+b+ce)_{t+1}/2) + 0.5 + mlogs/2))
  (sigmoid written as 0.5*tanh(x/2)+0.5 so tanh and exp share ONE ACT
  table set - no 1.3us table reloads). Emit-on-k keeps every per-(t,k)
  quantity row-major - no layout transposes anywhere on device.

  L1 (memory regime): the host stages the embedding table TRANSPOSED
  (256, V) in fp8-e4m3; each core streams its V/8 shard sequentially
  (3.2 MB) and computes proj = [g_t0|g_t1|g_e1]^T @ tableT with single
  fp8 DoubleRow matmuls (both 128-row k-tiles in one half-rate pass).
  obs is staged transposed too (bf16), giving the a-row the same way.
  (An on-device indirect row gather was tried instead: random 512B DMA
  descriptors run latency-bound at ~1.4 GB/s/engine, 30x slower than
  this sequential stream.)

  Host glue between launches: gathers proj[:, candidate_ids] (~50
  KB/core) and restages it - pure indexing, like the baseline.

  L2 (compute): per core, 128 S-leaves are built two groups at a time
  stacked across all 128 partitions: one K=16 matmul per 8 frames
  against host-staged block-diagonal [u;1]/[diag-ones;v] operands,
  full-width tanh, then exp writes straight into a zeroed block-diagonal
  "leafpair" buffer (pair = two subchains). The chain advances 32
  subchains of 4 leaves as 64 paired 128x128 bf16 matmuls; the PSUM
  evacuation of each round applies D as a free-dim-broadcast multiply.
  The host combines the 8*32 subchain matrices in f64 (as the baseline
  did).
"""

import numpy as np

T = 1024
K = 64
D = 256
V = 100000
NCORES = 8
VSH = 12544          # V-shard columns per core (8*12544 >= V)
NFR = 129            # frames per core (128 + 1 overlap)
NROW = 8320          # staged (t,k) rows: 129*64 = 8256, padded
NG = 16              # leaf groups of 8 frames
NSUB = 32            # subchains per core
LSUB = 4             # leaves per subchain
NBATCH = 4           # chain batches of 8 subchains
LTW = 2048           # L1 stream tile width (columns)

_PROG = {}


def _gvec(w3, l):
    g = np.zeros_like(l)
    g += w3[1] * l
    g[:-1] += w3[0] * l[1:]
    g[1:] += w3[2] * l[:-1]
    return g


def _mods():
    import sys
    if "/opt/trn_rl_repo" not in sys.path:
        sys.path.insert(0, "/opt/trn_rl_repo")
    import concourse.bacc as bacc
    import concourse.mybir as mybir
    from concourse import tile, bass
    return bacc, mybir, tile, bass


def _build_l1():
    if "l1" in _PROG:
        return _PROG["l1"]
    bacc, mybir, tile, bass = _mods()
    f32 = mybir.dt.float32
    bf16 = mybir.dt.bfloat16

    f8 = mybir.dt.float8e4
    nc = bacc.Bacc("TRN2", target_bir_lowering=False, debug=False,
                   enable_asserts=False, num_devices=NCORES)
    ttc = nc.dram_tensor("ttc", (D, VSH), f8, kind="ExternalInput").ap()
    obsT = nc.dram_tensor("obsT", (D, 256), bf16, kind="ExternalInput").ap()
    gmat = nc.dram_tensor("gmat", (D, 16), f8, kind="ExternalInput").ap()
    gobs = nc.dram_tensor("gobs", (D, 1), bf16, kind="ExternalInput").ap()
    projout = nc.dram_tensor("projout", (3, VSH), bf16,
                             kind="ExternalOutput").ap()
    arowout = nc.dram_tensor("arowout", (1, 256), f32,
                             kind="ExternalOutput").ap()

    ttr = ttc.rearrange("(c p) v -> p c v", p=128)
    with tile.TileContext(nc) as tc:
        with (
            tc.tile_pool(name="persist", bufs=1) as pp,
            tc.tile_pool(name="load", bufs=8) as lp,
            tc.tile_pool(name="out", bufs=3) as op,
            tc.tile_pool(name="ps_pj", bufs=4, space="PSUM") as ps_pj,
        ):
            # small inputs go via DVE's DMA queue: SP streams table tiles
            # immediately and ACT's queue is blocked by its table load
            g_sb = pp.tile([128, 2, 16], f8, tag="gmat")
            nc.gpsimd.dma_start(g_sb[:], gmat.rearrange("(c p) g -> p c g", p=128))
            go_sb = pp.tile([128, 2, 1], bf16, tag="gobs")
            nc.gpsimd.dma_start(go_sb[:], gobs.rearrange("(c p) g -> p c g", p=128))

            # a-row from transposed obs: 2 matmuls, no transposes
            obsT_sb = pp.tile([128, 2, 256], bf16, tag="obsT")
            nc.gpsimd.dma_start(obsT_sb[:],
                                obsT.rearrange("(c p) f -> p c f", p=128))
            arow_ps = ps_pj.tile([1, 256], f32, tag="ar")
            for ch in range(2):
                nc.tensor.matmul(
                    out=arow_ps[:], lhsT=go_sb[:, ch, :],
                    rhs=obsT_sb[:, ch, :], start=(ch == 0), stop=(ch == 1),
                )
            arow = pp.tile([1, 256], f32, tag="arow")
            nc.vector.tensor_copy(out=arow[:], in_=arow_ps[:])
            nc.sync.dma_start(out=arowout, in_=arow[:])

            # stream the tableT shard; proj rows [u, v, b].  The first two
            # tiles are small so compute starts as early as possible.
            widths = [1024, 1024] + [2048] * 5 + [256]
            los = [sum(widths[:i]) for i in range(len(widths))]
            for vt, (lo, w) in enumerate(zip(los, widths)):
                tt = lp.tile([128, 2, LTW], f8, tag="tt")
                nc.sync.dma_start(tt[:, :, :w], ttr[:, :, lo:lo + w])
                pr = op.tile([3, LTW], bf16, tag="pr")
                for j in range((w + 511) // 512):
                    w2 = min(512, w - j * 512)
                    pj = ps_pj.tile([16, 512], f32, tag="pj")
                    # fp8 DoubleRow: both 128-row k-tiles contract in one
                    # half-cycle-per-column pass
                    nc.tensor.matmul(
                        out=pj[:, :w2],
                        lhsT=g_sb[:],
                        rhs=tt[:, :, j * 512: j * 512 + w2],
                        start=True, stop=True,
                        perf_mode=mybir.MatmulPerfMode.DoubleRow,
                    )
                    if (vt + j) % 2 == 0:
                        nc.vector.tensor_copy(
                            out=pr[:, j * 512: j * 512 + w2], in_=pj[0:3, :w2])
                    else:
                        nc.scalar.copy(
                            out=pr[:, j * 512: j * 512 + w2], in_=pj[0:3, :w2])
                # issue on the idle Pool queue: SP would head-of-line block
                # the next tile load, ACT would stall the evacuations
                nc.gpsimd.dma_start(out=projout[:, lo:lo + w], in_=pr[:, :w])
    nc.compile()
    _PROG["l1"] = nc
    return nc


def _build_l2():
    if "l2" in _PROG:
        return _PROG["l2"]
    bacc, mybir, tile, bass = _mods()
    f32 = mybir.dt.float32
    bf16 = mybir.dt.bfloat16
    AF = mybir.ActivationFunctionType
    OP = mybir.AluOpType

    nc = bacc.Bacc("TRN2", target_bir_lowering=False, debug=False,
                   enable_asserts=False, num_devices=NCORES)
    ul_in = nc.dram_tensor("ulhsT", (2 * 8, NG * K), bf16,
                           kind="ExternalInput").ap()
    vr_in = nc.dram_tensor("vrhs", (2 * 8, NG * 512), bf16,
                           kind="ExternalInput").ap()
    emp_in = nc.dram_tensor("empreT", (128, K), bf16, kind="ExternalInput").ap()
    em0_in = nc.dram_tensor("em0pre", (1, K), f32, kind="ExternalInput").ap()
    cols_in = nc.dram_tensor("cols", (128, 4), f32, kind="ExternalInput").ap()
    addend = nc.dram_tensor("addend", (K, K), bf16, kind="ExternalInput").ap()
    qinit = nc.dram_tensor("qinit", (128, 256), bf16, kind="ExternalInput").ap()
    qout = nc.dram_tensor("qout", (128, NG * K), f32, kind="ExternalOutput").ap()
    em0out = nc.dram_tensor("em0out", (1, K), f32, kind="ExternalOutput").ap()

    with tile.TileContext(nc) as tc:
        with (
            tc.tile_pool(name="persist", bufs=1) as pp,
            tc.tile_pool(name="grp", bufs=3) as gp,
            tc.tile_pool(name="qq", bufs=3) as qp,
            tc.tile_pool(name="ps_pl", bufs=2, space="PSUM") as ps_pl,
            tc.tile_pool(name="ps_pq", bufs=2, space="PSUM") as ps_pq,
        ):
            # the leaf matmul operands gate everything - load them first on SP
            ulhsT0 = pp.tile([16, NG * K], bf16, tag="ulhsT")
            nc.sync.dma_start(ulhsT0[:], ul_in)
            vrhs0 = pp.tile([16, NG * 512], bf16, tag="vrhs")
            for c4 in range(4):
                nc.sync.dma_start(vrhs0[:, c4 * 2048:(c4 + 1) * 2048],
                                  vr_in[:, c4 * 2048:(c4 + 1) * 2048])

            cols = pp.tile([128, 4], f32, tag="cols")
            nc.scalar.dma_start(cols[:], cols_in)
            ct2_col = cols[:, 0:1]
            cp_col = cols[:, 1:2]       # 0.5 + mlogs/2
            mask_col = cols[:, 2:3]
            maski_col = cols[:, 3:4]    # 1 - mask

            # emit factor as a diagonal: d[k] = exp(0.5*tanh(empre/2) + c'),
            # staged transposed+paired by the host (k on partitions, one
            # column per (pair, round) = leaf)
            empreT = pp.tile([128, K], bf16, tag="empreT")
            nc.scalar.dma_start(empreT[:], emp_in)
            th2d = pp.tile([128, K], bf16, tag="th2d")
            nc.scalar.activation(th2d[:], empreT[:], AF.Tanh, scale=0.5)
            dmat = pp.tile([128, K], bf16, tag="dmat")
            nc.scalar.activation(dmat[:], th2d[:], AF.Exp, bias=cp_col,
                                 scale=0.5)
            # pad-leaf column: d -> 1 on the last core
            nc.vector.scalar_tensor_tensor(
                out=dmat[K:128, 63:64], in0=dmat[K:128, 63:64],
                scalar=mask_col[K:128, :], in1=maski_col[K:128, :],
                op0=OP.mult, op1=OP.add,
            )
            dmv = dmat[:].rearrange("p (m r) -> p m r", r=4)

            ulhsT = ulhsT0
            vrhs = vrhs0
            add_sb2 = pp.tile([128, K], bf16, tag="addend2")
            nc.scalar.dma_start(add_sb2[K:128, :], addend)

            # ---- emit_0 ----
            em0p = pp.tile([1, K], f32, tag="em0p")
            nc.sync.dma_start(em0p[:], em0_in)
            th0 = pp.tile([1, K], f32, tag="th0")
            nc.scalar.activation(th0[:], em0p[:], AF.Tanh, scale=0.5)
            em0sb = pp.tile([1, K], f32, tag="em0")
            nc.vector.tensor_scalar(out=em0sb[:], in0=th0[:], scalar1=0.5,
                                    scalar2=0.5, op0=OP.mult, op1=OP.add)
            nc.sync.dma_start(out=em0out, in_=em0sb[:])

            # zeroed block-diagonal leaf-pair buffer (off-quadrants stay 0).
            # Pair p: top = subchain 4*(p//2)+(p%2) (an even group), bottom =
            # that + 2 (the odd group of the same pair of groups); round r
            # blocks sit at cols (p*4+r)*128, top-left / bottom-right.
            leafpair = pp.tile([128, NG * 512], bf16, tag="leafpair")
            nc.gpsimd.memset(leafpair[:, :NG * 256], 0)
            nc.gpsimd.memset(leafpair[:, NG * 256:], 0)
            lpv = leafpair[:].rearrange("p (m r x) -> p m r x", r=4, x=128)

            # ---- S-leaves: exp(0.5*th1 + c') ----
            # Per pair of groups (2jp -> partitions 0:64, 2jp+1 -> 64:128):
            # two K=16 matmuls against host-staged block-diagonal [u;1] /
            # [diag-ones; v] operands fill one (128,512) psum; tanh runs
            # full-width; exp writes straight into the leafpair quadrants,
            # batched two group-pairs per instruction.
            th1big = pp.tile([128, 8 * 512], bf16, tag="th1big")
            for jp in range(8):
                pl = ps_pl.tile([128, 512], f32, tag="pl")
                nc.tensor.matmul(
                    out=pl[0:K, :],
                    lhsT=ulhsT[:, (2 * jp) * K:(2 * jp + 1) * K],
                    rhs=vrhs[:, (2 * jp) * 512:(2 * jp + 1) * 512],
                    start=True, stop=True,
                )
                nc.tensor.matmul(
                    out=pl[K:128, :],
                    lhsT=ulhsT[:, (2 * jp + 1) * K:(2 * jp + 2) * K],
                    rhs=vrhs[:, (2 * jp + 1) * 512:(2 * jp + 2) * 512],
                    start=True, stop=True,
                )
                nc.scalar.activation(th1big[:, jp * 512:(jp + 1) * 512],
                                     pl[:], AF.Tanh, bias=ct2_col, scale=0.5)
                if jp % 2 == 1:
                    s2v = th1big[:, (jp - 1) * 512:(jp + 1) * 512].rearrange(
                        "p (a r k) -> p a r k", r=4, k=K)
                    p0 = 2 * (jp - 1)
                    if jp < 7:
                        nc.scalar.activation(
                            lpv[0:K, p0:p0 + 4, :, 0:K],
                            s2v[0:K, :, :, :], AF.Exp, bias=cp_col[0:K, :],
                            scale=0.5)
                        nc.scalar.activation(
                            lpv[K:128, p0:p0 + 4, :, K:128],
                            s2v[K:128, :, :, :], AF.Exp, bias=cp_col[K:128, :],
                            scale=0.5)
                    else:
                        # last batch: emit per-round so its chain rounds can
                        # start while later rounds' exps still run
                        for r in range(LSUB):
                            nc.scalar.activation(
                                lpv[0:K, p0:p0 + 4, r, 0:K],
                                s2v[0:K, :, r, :], AF.Exp,
                                bias=cp_col[0:K, :], scale=0.5)
                            nc.scalar.activation(
                                lpv[K:128, p0:p0 + 4, r, K:128],
                                s2v[K:128, :, r, :], AF.Exp,
                                bias=cp_col[K:128, :], scale=0.5)

            # pad leaf 127 (pair 15 bottom, round 3) -> mask*leaf + addend
            last = lpv[K:128, 15, 3, K:128]
            nc.vector.scalar_tensor_tensor(
                out=last, in0=last, scalar=mask_col[K:128, :],
                in1=add_sb2[K:128, :], op0=OP.mult, op1=OP.add,
            )

            # ---- chain: 4 batches of 4 pairs (8 subchains), 4 rounds ----
            # the PSUM->SBUF evacuation applies the emit diagonal: Q <- D S^T Q
            qout_sb = pp.tile([128, NG * K], f32, tag="qout_sb")
            for b in range(NBATCH):
                qcur = qp.tile([128, 256], bf16, tag="q")
                nc.gpsimd.dma_start(qcur[:], qinit)
                for r in range(LSUB):
                    pq = ps_pq.tile([128, 256], f32, tag="pq")
                    for p in range(4):
                        m = 4 * b + p
                        nc.tensor.matmul(
                            out=pq[:, p * K:(p + 1) * K],
                            lhsT=lpv[:, m, r, :],
                            rhs=qcur[:, p * K:(p + 1) * K],
                            start=True, stop=True,
                        )
                    din = dmv[:, 4 * b:4 * b + 4, r].unsqueeze(2).to_broadcast(
                        (128, 4, K))
                    if r < LSUB - 1:
                        qnext = qp.tile([128, 256], bf16, tag="q")
                        dst = qnext[:]
                    else:
                        dst = qout_sb[:, b * 256:(b + 1) * 256]
                    nc.vector.tensor_tensor(
                        out=dst.rearrange("p (q k) -> p q k", k=K),
                        in0=pq[:].rearrange("p (q k) -> p q k", k=K),
                        in1=din, op=OP.mult,
                    )
                    if r < LSUB - 1:
                        qcur = qnext
            nc.sync.dma_start(out=qout, in_=qout_sb[:])
    nc.compile()
    _PROG["l2"] = nc
    return nc


def _host_consts(inputs):
    E = np.asarray(inputs["word_embeds"], dtype=np.float32)
    ids = np.asarray(inputs["candidate_ids"]).astype(np.int64)
    obs = np.ascontiguousarray(np.asarray(inputs["observed_feats"], dtype=np.float32))

    lw_e = np.asarray(inputs["emit_lin_w"], dtype=np.float64)[0]
    lw_t = np.asarray(inputs["trans_lin_w"], dtype=np.float64)[0]
    cw_e = np.asarray(inputs["emit_conv_w"], dtype=np.float64)
    cw_t = np.asarray(inputs["trans_conv_w"], dtype=np.float64)
    g_e0 = _gvec(cw_e[0, 0], lw_e)
    g_e1 = _gvec(cw_e[0, 1], lw_e)
    g_t0 = _gvec(cw_t[0, 0], lw_t)
    g_t1 = _gvec(cw_t[0, 1], lw_t)
    ce = float(np.asarray(inputs["emit_conv_b"], np.float64)[0] * lw_e.sum()
               + np.asarray(inputs["emit_lin_b"], np.float64)[0])
    ct = float(np.asarray(inputs["trans_conv_b"], np.float64)[0] * lw_t.sum()
               + np.asarray(inputs["trans_lin_b"], np.float64)[0])

    samp = E[ids[:8].ravel()].astype(np.float64)
    sig = 1.0 / (1.0 + np.exp(-((samp @ g_t0).mean() + (samp @ g_t1).mean() + ct)))
    a8 = obs[:8].astype(np.float64) @ g_e0
    em = 1.0 / (1.0 + np.exp(-(a8.mean() + (samp @ g_e1).mean() + ce)))
    s = float(64.0 * np.exp(sig + em))
    gmat = np.stack([g_t0, g_t1, g_e1, g_e0, np.zeros(D)], axis=1)
    return E, ids, obs, gmat, ce, ct, s


def _run_launches(inputs, run_kw1=None, run_kw2=None):
    import sys
    if "/opt/trn_rl_repo" not in sys.path:
        sys.path.insert(0, "/opt/trn_rl_repo")
    from concourse.bass_utils import run_bass_kernel_spmd
    import ml_dtypes

    bf16 = ml_dtypes.bfloat16
    run_kw1 = run_kw1 or {}
    run_kw2 = run_kw2 or {}
    E, ids, obs, gmat, ce, ct, s = _host_consts(inputs)
    mlogs = -np.log(s)

    # ---- L1: stream transposed table shards (fp8) ----
    f8 = ml_dtypes.float8_e4m3
    tT = np.zeros((D, NCORES * VSH), dtype=f8)
    tT[:, :V] = np.ascontiguousarray(E.T.astype(f8))
    obsTf = np.zeros((D, T + 128), dtype=bf16)
    obsTf[:, :T] = np.ascontiguousarray(obs.T.astype(bf16))
    gmb = np.ascontiguousarray(
        np.concatenate([gmat[:, 0:3], np.zeros((D, 13))], axis=1)
        .astype(np.float32).astype(f8))
    gob = np.ascontiguousarray(gmat[:, 3:4].astype(np.float32).astype(bf16))

    l1 = _build_l1()
    in1 = [{
        "ttc": np.ascontiguousarray(tT[:, c * VSH:(c + 1) * VSH]),
        "obsT": np.ascontiguousarray(obsTf[:, 128 * c: 128 * c + 256]),
        "gmat": gmb,
        "gobs": gob,
    } for c in range(NCORES)]
    res1 = run_bass_kernel_spmd(l1, in1, core_ids=list(range(NCORES)),
                                **run_kw1)
    proj = np.concatenate(
        [res1.results[c]["projout"] for c in range(NCORES)], axis=1)[:, :V]

    # ---- host gather + staging (indexing glue) ----
    ids_pad = np.zeros((T + 1, K), dtype=np.int64)
    ids_pad[:T] = ids
    l2 = _build_l2()
    eye_s = (np.eye(K, dtype=np.float64) / s).astype(np.float32).astype(bf16)
    zer = np.zeros((K, K), dtype=bf16)
    eye2 = np.vstack([np.eye(K, dtype=np.float32)] * 2)
    qinitb = np.ascontiguousarray(np.tile(eye2, (1, 4)).astype(bf16))
    in2 = []
    for c in range(NCORES):
        fr0 = 128 * c
        rid = ids_pad[fr0:fr0 + NFR].ravel()          # 8256
        pg = proj[:, rid]                              # (3, 8256) bf16
        u = pg[0].astype(np.float32)
        v = pg[1].astype(np.float32)
        # block-diagonal outer-sum operands, one K=16 matmul per 8 frames
        ulhsT = np.zeros((16, NG * K), dtype=np.float32)
        vrhs = np.zeros((16, NG * 512), dtype=np.float32)
        ur = u[:8192].reshape(NG, 8, K)
        vr = v[K:8256].reshape(NG, 8, K)               # frames 1..128
        vrh = vrhs.reshape(16, NG, 8, K)
        ulh = ulhsT.reshape(16, NG, K)
        for q in range(8):
            ulh[2 * q] = ur[:, q, :]
            ulh[2 * q + 1] = 1.0
            vrh[2 * q, :, q, :] = 1.0
            vrh[2 * q + 1, :, q, :] = vr[:, q, :]
        # emit pre-activations: restage L1's b/a outputs with the +ce fold.
        # empreT[(h*64+k), p*4+r] = empre of the leaf at (pair p, round r,
        # half h), where pair p tops subchain 4*(p//2)+(p%2), bottom +2.
        arow_c = res1.results[c]["arowout"][0].astype(np.float32)
        bfl = pg[2].astype(np.float32)                 # (8256,) b dots
        empre = (bfl[K:8256].reshape(128, K)
                 + arow_c[1:129, None] + np.float32(ce))   # (128 leaves, K)
        empreT = np.zeros((128, K), dtype=np.float32)
        for p in range(16):
            sc_t = 4 * (p // 2) + (p % 2)
            for r in range(4):
                empreT[0:K, p * 4 + r] = empre[4 * sc_t + r]
                empreT[K:128, p * 4 + r] = empre[4 * (sc_t + 2) + r]
        em0pre = (bfl[:K] + arow_c[0] + np.float32(ce)).reshape(1, K)
        colsv = np.zeros((128, 4), dtype=np.float32)
        colsv[:, 0] = ct / 2.0
        colsv[:, 1] = 0.5 + mlogs / 2.0
        colsv[:, 2] = 0.0 if c == NCORES - 1 else 1.0
        colsv[:, 3] = 1.0 - colsv[:, 2]
        in2.append({
            "ulhsT": np.ascontiguousarray(ulhsT.astype(bf16)),
            "vrhs": np.ascontiguousarray(vrhs.astype(bf16)),
            "empreT": np.ascontiguousarray(empreT.astype(bf16)),
            "em0pre": np.ascontiguousarray(em0pre),
            "cols": colsv,
            "addend": eye_s if c == NCORES - 1 else zer,
            "qinit": qinitb,
        })
    res2 = run_bass_kernel_spmd(l2, in2, core_ids=list(range(NCORES)),
                                **run_kw2)

    # ---- host combine in f64 ----
    # qout is (128, 16*64): pair p (cols p*64) holds subchain 4*(p//2)+(p%2)
    # in rows 0:64 and that + 2 in rows 64:128
    P = np.eye(K, dtype=np.float64)
    acc = 0.0
    for c in range(NCORES):
        qo = res2.results[c]["qout"].astype(np.float64)
        for sc in range(NSUB):
            blk, rem = divmod(sc, 4)
            h, p = (0, 2 * blk + rem) if rem < 2 else (1, 2 * blk + rem - 2)
            Q = qo[h * K:(h + 1) * K, p * K:(p + 1) * K]
            P = P @ Q.T
            mx = np.abs(P).max()
            P /= mx
            acc += np.log(mx)
    em0 = res2.results[0]["em0out"][0].astype(np.float64)
    z = np.exp(em0) @ P @ np.ones(K)
    ans = np.log(z) + acc + NSUB * LSUB * NCORES * np.log(np.float64(s))
    return np.array([ans], dtype=np.float32), res1, res2


def kernel(**inputs):
    ans, _, _ = _run_launches(inputs)
    return ans


def profiled_run(inputs):
    """Run both launches with NTFF tracing; return summed exec ns (or None)."""
    import sys as _sys
    import types as _types
    try:
        if "antenv.axon_hooks" not in _sys.modules:
            from trn_agent_boot.trn_boot import _ntff_profile_via_ctypes
            hook = _ntff_profile_via_ctypes("/opt/axon/libaxon_pjrt.so")
            mod = _types.ModuleType("antenv.axon_hooks")
            mod.get_axon_ntff_profile_hook = lambda: hook
            mod.set_axon_ntff_profile_hook = lambda h: None
            _sys.modules["antenv.axon_hooks"] = mod
            import antenv
            antenv.axon_hooks = mod
    except Exception as e:
        print(f"profile shim unavailable: {e}")
        return None
    kw = {"trace": True, "trace_cores": [0]}
    ans, res1, res2 = _run_launches(inputs, run_kw1=dict(kw), run_kw2=dict(kw))
    print("profiled answer:", ans)
    for name, r in (("L1", res1), ("L2", res2)):
        tr = r.instructions_and_trace
        print(f"{name}: exec_time_ns={r.exec_time_ns}"
              + (f" trace={tr[1]}" if tr else ""))
    if res1.exec_time_ns is None or res2.exec_time_ns is None:
        return None
    return res1.exec_time_ns + res2.exec_time_ns
